# revision 15
# baseline (speedup 1.0000x reference)
"""Block-quantized FP8 linear (KLinearFP8) on 8 trn2 NeuronCores.

y[m, n] = sum_k x_dq[m, k] * w_dq[n, k]
  x_dq: per-(row, 128-block) fp8e4m3fn-simulated quantization of x
  w_dq: weight (fp8 values held in fp32) * per-128x128-block scale

Sharding: column-parallel. weight/weight_scale_inv split along N across 8
cores, x replicated; each core computes y[:, c*2048:(c+1)*2048].

Weight path (the one change vs the proven pipeline): the weight shard
ships host-transposed [K, NSH] as TRN-safe fp8 (w/2 in
ml_dtypes.float8_e4m3 — all values <=224, lossless cast; layout/dtype
transform only). Each k-slab is one contiguous DMA straight into the
K-on-partitions layout the PE needs, dequantized to bf16 with a single
multiply (2*ws folded in). The tensor engine therefore runs ONLY the
GEMM — no PE-array transposes, no weight-prep phase ahead of the matmul
stream.

Per-core x path: quantize+dequantize x per (row, 128-block) with scale
amax/224 (power-of-two rescale of the reference amax/448 grid ->
identical rounding), XBAR-transpose to K-on-partitions, bf16 GEMM with
fp32 PSUM accumulation.
"""

import numpy as np

M, K, N = 4096, 4096, 16384
NCORES = 8
NSH = N // NCORES          # 2048 columns of y per core
P = 128
KB = K // P                # 32 k-blocks
KH = KB // 2               # 16 k-blocks per half (SBUF fit)
MT = M // P                # 32 m-tiles
NB = NSH // P              # 16 n-blocks per core
NCH = NSH // 512           # 4 psum chunks of 512
CHW = 512
FP8_SAFE = 224.0           # 448/2: fits TRN e4m3 (max 240), same rounding grid

_NC_CACHE = {}


def _build(M=M, K=K, NSH=NSH, debug=False):
    import concourse.bass as bass  # noqa: F401
    import concourse.mybir as mybir
    import concourse.tile as tile
    from concourse import bacc

    KB = K // P
    KH = KB // 2
    MT = M // P
    NB = NSH // P
    CHW = min(512, NSH)
    NCH = NSH // CHW

    f32, bf16, f8 = mybir.dt.float32, mybir.dt.bfloat16, mybir.dt.float8e4

    nc = bacc.Bacc(None, target_bir_lowering=False, debug=debug)
    x_d = nc.declare_dram_parameter("x", [M, K], f32, isOutput=False)
    wt8_d = nc.declare_dram_parameter("wt8", [K, NSH], f8, isOutput=False)
    ws_d = nc.declare_dram_parameter("ws", [NB, KB], f32, isOutput=False)
    y_d = nc.declare_dram_parameter("y", [M, NSH], bf16, isOutput=True)

    with tile.TileContext(nc) as tc:
        with (
            tc.tile_pool(name="const", bufs=1) as const,
            tc.tile_pool(name="wt", bufs=1) as wtp,
            tc.tile_pool(name="w8p", bufs=2) as w8p,
            tc.tile_pool(name="xpool", bufs=2) as xpool,
            tc.tile_pool(name="xtp", bufs=6) as xtp,
            tc.tile_pool(name="scales", bufs=3) as spool,
            tc.tile_pool(name="ypool", bufs=4) as ypool,
            tc.tile_pool(name="psum", bufs=8, space="PSUM") as psum,
        ):
            # ---- weight-block scales (shipped pre-doubled: host sends
            # 2*ws to undo the /2 fp8 encoding), replicated to all
            # partitions with a single 0-stride-source DMA -- no engine
            # hop gating the weight dequants.
            wsb = const.tile([P, NB, KB], f32)
            nc.sync.dma_start(
                wsb[:],
                ws_d[None, :, :].to_broadcast((P, NB, KB)),
            )

            # ---- weight prep: one contiguous DMA per k-slab (already
            # K-on-partitions), one dequant multiply to bf16. No PE work.
            # Dequants split DVE/GpSimd so neither engine's FIFO backlog
            # starves the first m-tiles' x-prep.
            wTs = [None] * KB

            # ---- x-prep for one m-tile: quantize+dequantize (two
            # k-halves), XBAR-transpose to K-on-partitions.
            def x_prep(mt):
                ms = slice(mt * P, (mt + 1) * P)
                xThalf = []
                for kh in range(2):
                    ks = slice(kh * KH * P, (kh + 1) * KH * P)
                    xrow = xpool.tile([P, KH, P], f32, name="xrow", tag="xrow")
                    nc.scalar.dma_start(
                        xrow[:],
                        x_d[ms, ks].rearrange("m (kb x) -> m kb x", x=P),
                    )
                    sc = spool.tile([P, 3, KH], f32, name="sc", tag="sc")
                    amax, rinv, s2 = sc[:, 0, :], sc[:, 1, :], sc[:, 2, :]
                    nc.vector.tensor_reduce(
                        amax, xrow[:], axis=mybir.AxisListType.X,
                        op=mybir.AluOpType.max, apply_absolute_value=True,
                    )
                    nc.vector.reciprocal(rinv, amax)
                    nc.vector.tensor_scalar_mul(rinv, rinv, float(FP8_SAFE))
                    nc.vector.tensor_scalar_mul(s2, amax, float(1.0 / FP8_SAFE))
                    xq = xpool.tile([P, KH, P], f8, name="xq", tag="xq")
                    nc.vector.tensor_tensor(
                        xq[:], xrow[:], rinv[:, :, None].to_broadcast((P, KH, P)),
                        mybir.AluOpType.mult,
                    )
                    xdq = xpool.tile([P, KH, P], bf16, name="xdq", tag="xdq")
                    nc.vector.tensor_tensor(
                        xdq[:], xq[:], s2[:, :, None].to_broadcast((P, KH, P)),
                        mybir.AluOpType.mult,
                    )
                    xT = xtp.tile([P, KH, P], bf16, name="xT", tag="xT")
                    nc.sync.dma_start_transpose(
                        xT[:], xdq[:].rearrange("p a b -> p (a b)")
                    )
                    xThalf.append(xT)
                return xThalf

            def drains(mt, pts):
                ms = slice(mt * P, (mt + 1) * P)
                for c in range(NCH):
                    yt = ypool.tile([P, CHW], bf16, name="yt", tag="yt")
                    nc.scalar.activation(
                        yt[:], pts[c][:], mybir.ActivationFunctionType.Copy
                    )
                    # y via SWDGE keeps HWDGE lanes clear for x loads +
                    # transposes.
                    nc.gpsimd.dma_start(y_d[ms, c * CHW:(c + 1) * CHW], yt[:])

            # ---- first m-tile's x-prep is emitted before the weight
            # loop so its loads aren't queued behind 32 w8 DMA triggers.
            xT_first = x_prep(0)
            for kb in range(KB):
                w8 = w8p.tile([P, NB, P], f8, name="w8", tag="w8")
                # sync queue: keeps the scalar engine's HWDGE ring free
                # for the per-m-tile x loads (32 queued triggers would
                # delay mt1+'s prep by ~20us).
                nc.sync.dma_start(
                    w8[:].rearrange("p a b -> p (a b)"),
                    wt8_d[kb * P:(kb + 1) * P, :],
                )
                wT = wtp.tile([P, NB, P], bf16, name="wT", tag=f"wT{kb}")
                eng = nc.vector if kb % 16 < 8 else nc.gpsimd
                eng.tensor_tensor(
                    wT[:], w8[:],
                    wsb[:, :, kb, None].to_broadcast((P, NB, P)),
                    mybir.AluOpType.mult,
                )
                wTs[kb] = wT

            # ---- software-pipelined main loop: x-prep one m-tile ahead,
            # drains one m-tile behind (their matmul-completion waits are
            # then pre-satisfied and never block the scalar queue).
            xT_next = xT_first
            prev = None
            for mt in range(MT):
                xThalf = xT_next
                if mt + 1 < MT:
                    xT_next = x_prep(mt + 1)
                if prev is not None:
                    drains(*prev)
                pts = [
                    psum.tile([P, CHW], mybir.dt.float32, name=f"pt{c}", tag="pt")
                    for c in range(NCH)
                ]
                for kh in range(2):
                    for c in range(NCH):
                        for kb in range(KH):
                            wv = wTs[kh * KH + kb][:].rearrange("p a b -> p (a b)")
                            nc.tensor.matmul(
                                pts[c][:],
                                xThalf[kh][:, kb, :],
                                wv[:, c * CHW:(c + 1) * CHW],
                                start=(kh == 0 and kb == 0),
                                stop=(kh == 1 and kb == KH - 1),
                            )
                prev = (mt, pts)
            drains(*prev)

    nc.compile()
    return nc


def _core_inputs(x, weight, ws, c, nsh=NSH, nb=NB):
    """Shard + lay out inputs for core c. Layout/dtype transforms only:
    the fp8 cast of w/2 is exact (all values <= 224)."""
    import ml_dtypes

    wsl = weight[c * nsh:(c + 1) * nsh]
    wt8 = np.ascontiguousarray(
        (wsl.T * np.float32(0.5)).astype(ml_dtypes.float8_e4m3)
    )
    return {
        "x": x,
        "wt8": wt8,
        "ws": np.ascontiguousarray(ws[c * nb:(c + 1) * nb] * np.float32(2.0)),
    }


def kernel(x, weight, weight_scale_inv):
    from concourse.bass_utils import run_bass_kernel_spmd

    if "nc" not in _NC_CACHE:
        _NC_CACHE["nc"] = _build()
    nc = _NC_CACHE["nc"]

    x = np.ascontiguousarray(np.asarray(x, dtype=np.float32))
    weight = np.asarray(weight, dtype=np.float32)
    ws = np.asarray(weight_scale_inv, dtype=np.float32)

    in_maps = [_core_inputs(x, weight, ws, c) for c in range(NCORES)]
    res = run_bass_kernel_spmd(nc, in_maps, list(range(NCORES)))
    y = np.concatenate(
        [np.asarray(res.results[c]["y"]) for c in range(NCORES)], axis=1
    )
    return y.astype(np.float32, copy=False)


# revision 16
# speedup vs baseline: 1.0450x; 1.0450x over previous
"""Block-quantized FP8 linear (KLinearFP8) on 8 trn2 NeuronCores.

y[m, n] = sum_k x_dq[m, k] * w_dq[n, k]
  x_dq: per-(row, 128-block) fp8e4m3fn-simulated quantization of x
  w_dq: weight (fp8 values held in fp32) * per-128x128-block scale

Sharding: column-parallel. weight/weight_scale_inv split along N across 8
cores, x replicated; each core computes y[:, c*2048:(c+1)*2048].

Weight path (the one change vs the proven pipeline): the weight shard
ships host-transposed [K, NSH] as TRN-safe fp8 (w/2 in
ml_dtypes.float8_e4m3 — all values <=224, lossless cast; layout/dtype
transform only). Each k-slab is one contiguous DMA straight into the
K-on-partitions layout the PE needs, dequantized to bf16 with a single
multiply (2*ws folded in). The tensor engine therefore runs ONLY the
GEMM — no PE-array transposes, no weight-prep phase ahead of the matmul
stream.

Per-core x path: quantize+dequantize x per (row, 128-block) with scale
amax/224 (power-of-two rescale of the reference amax/448 grid ->
identical rounding), XBAR-transpose to K-on-partitions, bf16 GEMM with
fp32 PSUM accumulation.
"""

import numpy as np

M, K, N = 4096, 4096, 16384
NCORES = 8
NSH = N // NCORES          # 2048 columns of y per core
P = 128
KB = K // P                # 32 k-blocks
KH = KB // 2               # 16 k-blocks per half (SBUF fit)
MT = M // P                # 32 m-tiles
NB = NSH // P              # 16 n-blocks per core
NCH = NSH // 512           # 4 psum chunks of 512
CHW = 512
FP8_SAFE = 224.0           # 448/2: fits TRN e4m3 (max 240), same rounding grid

_NC_CACHE = {}


def _build(M=M, K=K, NSH=NSH, debug=False):
    import concourse.bass as bass  # noqa: F401
    import concourse.mybir as mybir
    import concourse.tile as tile
    from concourse import bacc

    KB = K // P
    KH = KB // 2
    MT = M // P
    NB = NSH // P
    CHW = min(512, NSH)
    NCH = NSH // CHW

    f32, bf16, f8 = mybir.dt.float32, mybir.dt.bfloat16, mybir.dt.float8e4

    nc = bacc.Bacc(None, target_bir_lowering=False, debug=debug)
    x_d = nc.declare_dram_parameter("x", [M, K], f32, isOutput=False)
    wt8_d = nc.declare_dram_parameter("wt8", [K, NSH], f8, isOutput=False)
    ws_d = nc.declare_dram_parameter("ws", [NB, KB], f32, isOutput=False)
    y_d = nc.declare_dram_parameter("y", [M, NSH], bf16, isOutput=True)

    with tile.TileContext(nc) as tc:
        with (
            tc.tile_pool(name="const", bufs=1) as const,
            tc.tile_pool(name="wt", bufs=1) as wtp,
            tc.tile_pool(name="w8p", bufs=2) as w8p,
            tc.tile_pool(name="xpool", bufs=2) as xpool,
            tc.tile_pool(name="xtp", bufs=6) as xtp,
            tc.tile_pool(name="scales", bufs=3) as spool,
            tc.tile_pool(name="ypool", bufs=4) as ypool,
            tc.tile_pool(name="psum", bufs=8, space="PSUM") as psum,
        ):
            # ---- weight-block scales * 2 (undoes the host /2),
            # broadcast to all partitions: wsb[p, nb, kb] = 2*ws[nb, kb].
            ws_row = const.tile([1, NB * KB], f32)
            nc.sync.dma_start(
                ws_row[:], ws_d[:].rearrange("a b -> (a b)")[None, :]
            )
            nc.vector.tensor_scalar_mul(ws_row[:], ws_row[:], 2.0)
            wsb = const.tile([P, NB, KB], f32)
            nc.gpsimd.partition_broadcast(
                wsb[:].rearrange("p a b -> p (a b)"), ws_row[:]
            )

            # ---- weight prep: one contiguous DMA per k-slab (already
            # K-on-partitions), one dequant multiply to bf16. No PE work.
            # Dequants split DVE/GpSimd so neither engine's FIFO backlog
            # starves the first m-tiles' x-prep.
            wTs = [None] * KB

            # ---- x-prep for one m-tile: quantize+dequantize (two
            # k-halves), XBAR-transpose to K-on-partitions.
            def x_prep(mt):
                ms = slice(mt * P, (mt + 1) * P)
                xThalf = []
                for kh in range(2):
                    ks = slice(kh * KH * P, (kh + 1) * KH * P)
                    xrow = xpool.tile([P, KH, P], f32, name="xrow", tag="xrow")
                    nc.scalar.dma_start(
                        xrow[:],
                        x_d[ms, ks].rearrange("m (kb x) -> m kb x", x=P),
                    )
                    sc = spool.tile([P, 3, KH], f32, name="sc", tag="sc")
                    amax, rinv, s2 = sc[:, 0, :], sc[:, 1, :], sc[:, 2, :]
                    nc.vector.tensor_reduce(
                        amax, xrow[:], axis=mybir.AxisListType.X,
                        op=mybir.AluOpType.max, apply_absolute_value=True,
                    )
                    nc.vector.reciprocal(rinv, amax)
                    nc.vector.tensor_scalar_mul(rinv, rinv, float(FP8_SAFE))
                    nc.vector.tensor_scalar_mul(s2, amax, float(1.0 / FP8_SAFE))
                    xq = xpool.tile([P, KH, P], f8, name="xq", tag="xq")
                    nc.vector.tensor_tensor(
                        xq[:], xrow[:], rinv[:, :, None].to_broadcast((P, KH, P)),
                        mybir.AluOpType.mult,
                    )
                    xdq = xpool.tile([P, KH, P], bf16, name="xdq", tag="xdq")
                    nc.vector.tensor_tensor(
                        xdq[:], xq[:], s2[:, :, None].to_broadcast((P, KH, P)),
                        mybir.AluOpType.mult,
                    )
                    xT = xtp.tile([P, KH, P], bf16, name="xT", tag="xT")
                    nc.sync.dma_start_transpose(
                        xT[:], xdq[:].rearrange("p a b -> p (a b)")
                    )
                    xThalf.append(xT)
                return xThalf

            def drains(mt, pts):
                ms = slice(mt * P, (mt + 1) * P)
                for c in range(NCH):
                    yt = ypool.tile([P, CHW], bf16, name="yt", tag="yt")
                    nc.scalar.activation(
                        yt[:], pts[c][:], mybir.ActivationFunctionType.Copy
                    )
                    # y via SWDGE keeps HWDGE lanes clear for x loads +
                    # transposes.
                    nc.gpsimd.dma_start(y_d[ms, c * CHW:(c + 1) * CHW], yt[:])

            # ---- first m-tile's x-prep is emitted before the weight
            # loop so its loads aren't queued behind 32 w8 DMA triggers.
            xT_first = x_prep(0)
            for kb in range(KB):
                w8 = w8p.tile([P, NB, P], f8, name="w8", tag="w8")
                # sync queue: keeps the scalar engine's HWDGE ring free
                # for the per-m-tile x loads (32 queued triggers would
                # delay mt1+'s prep by ~20us).
                nc.sync.dma_start(
                    w8[:].rearrange("p a b -> p (a b)"),
                    wt8_d[kb * P:(kb + 1) * P, :],
                )
                wT = wtp.tile([P, NB, P], bf16, name="wT", tag=f"wT{kb}")
                eng = nc.vector if kb % 16 < 8 else nc.gpsimd
                eng.tensor_tensor(
                    wT[:], w8[:],
                    wsb[:, :, kb, None].to_broadcast((P, NB, P)),
                    mybir.AluOpType.mult,
                )
                wTs[kb] = wT

            # ---- software-pipelined main loop: x-prep one m-tile ahead,
            # drains one m-tile behind (their matmul-completion waits are
            # then pre-satisfied and never block the scalar queue).
            xT_next = xT_first
            prev = None
            for mt in range(MT):
                xThalf = xT_next
                if mt + 1 < MT:
                    xT_next = x_prep(mt + 1)
                if prev is not None:
                    drains(*prev)
                pts = [
                    psum.tile([P, CHW], mybir.dt.float32, name=f"pt{c}", tag="pt")
                    for c in range(NCH)
                ]
                for kh in range(2):
                    for c in range(NCH):
                        for kb in range(KH):
                            wv = wTs[kh * KH + kb][:].rearrange("p a b -> p (a b)")
                            nc.tensor.matmul(
                                pts[c][:],
                                xThalf[kh][:, kb, :],
                                wv[:, c * CHW:(c + 1) * CHW],
                                start=(kh == 0 and kb == 0),
                                stop=(kh == 1 and kb == KH - 1),
                            )
                prev = (mt, pts)
            drains(*prev)

    nc.compile()
    return nc


def _core_inputs(x, weight, ws, c, nsh=NSH, nb=NB):
    """Shard + lay out inputs for core c. Layout/dtype transforms only:
    the fp8 cast of w/2 is exact (all values <= 224)."""
    import ml_dtypes

    wsl = weight[c * nsh:(c + 1) * nsh]
    wt8 = np.ascontiguousarray(
        (wsl.T * np.float32(0.5)).astype(ml_dtypes.float8_e4m3)
    )
    return {
        "x": x,
        "wt8": wt8,
        "ws": np.ascontiguousarray(ws[c * nb:(c + 1) * nb]),
    }


def kernel(x, weight, weight_scale_inv):
    from concourse.bass_utils import run_bass_kernel_spmd

    if "nc" not in _NC_CACHE:
        _NC_CACHE["nc"] = _build()
    nc = _NC_CACHE["nc"]

    x = np.ascontiguousarray(np.asarray(x, dtype=np.float32))
    weight = np.asarray(weight, dtype=np.float32)
    ws = np.asarray(weight_scale_inv, dtype=np.float32)

    in_maps = [_core_inputs(x, weight, ws, c) for c in range(NCORES)]
    res = run_bass_kernel_spmd(nc, in_maps, list(range(NCORES)))
    y = np.concatenate(
        [np.asarray(res.results[c]["y"]) for c in range(NCORES)], axis=1
    )
    return y.astype(np.float32, copy=False)


# revision 17
# speedup vs baseline: 1.0620x; 1.0163x over previous
"""Block-quantized FP8 linear (KLinearFP8) on 8 trn2 NeuronCores.

y[m, n] = sum_k x_dq[m, k] * w_dq[n, k]
  x_dq: per-(row, 128-block) fp8e4m3fn-simulated quantization of x
  w_dq: weight (fp8 values held in fp32) * per-128x128-block scale

Sharding: column-parallel. weight/weight_scale_inv split along N across 8
cores, x replicated; each core computes y[:, c*2048:(c+1)*2048].

Weight path (the one change vs the proven pipeline): the weight shard
ships host-transposed [K, NSH] as TRN-safe fp8 (w/2 in
ml_dtypes.float8_e4m3 — all values <=224, lossless cast; layout/dtype
transform only). Each k-slab is one contiguous DMA straight into the
K-on-partitions layout the PE needs, dequantized to bf16 with a single
multiply (2*ws folded in). The tensor engine therefore runs ONLY the
GEMM — no PE-array transposes, no weight-prep phase ahead of the matmul
stream.

Per-core x path: quantize+dequantize x per (row, 128-block) with scale
amax/224 (power-of-two rescale of the reference amax/448 grid ->
identical rounding), XBAR-transpose to K-on-partitions, bf16 GEMM with
fp32 PSUM accumulation.
"""

import numpy as np

M, K, N = 4096, 4096, 16384
NCORES = 8
NSH = N // NCORES          # 2048 columns of y per core
P = 128
KB = K // P                # 32 k-blocks
KH = KB // 2               # 16 k-blocks per half (SBUF fit)
MT = M // P                # 32 m-tiles
NB = NSH // P              # 16 n-blocks per core
NCH = NSH // 512           # 4 psum chunks of 512
CHW = 512
FP8_SAFE = 224.0           # 448/2: fits TRN e4m3 (max 240), same rounding grid

_NC_CACHE = {}


def _build(M=M, K=K, NSH=NSH, debug=False):
    import concourse.bass as bass  # noqa: F401
    import concourse.mybir as mybir
    import concourse.tile as tile
    from concourse import bacc

    KB = K // P
    KH = KB // 2
    MT = M // P
    NB = NSH // P
    CHW = min(512, NSH)
    NCH = NSH // CHW

    f32, bf16, f8 = mybir.dt.float32, mybir.dt.bfloat16, mybir.dt.float8e4

    nc = bacc.Bacc(None, target_bir_lowering=False, debug=debug)
    x_d = nc.declare_dram_parameter("x", [M, K], f32, isOutput=False)
    wt8_d = nc.declare_dram_parameter("wt8", [K, NSH], f8, isOutput=False)
    ws_d = nc.declare_dram_parameter("ws", [NB, KB], f32, isOutput=False)
    y_d = nc.declare_dram_parameter("y", [M, NSH], bf16, isOutput=True)

    with tile.TileContext(nc) as tc:
        with (
            tc.tile_pool(name="const", bufs=1) as const,
            tc.tile_pool(name="wt", bufs=1) as wtp,
            tc.tile_pool(name="w8p", bufs=2) as w8p,
            tc.tile_pool(name="xpool", bufs=2) as xpool,
            tc.tile_pool(name="xtp", bufs=6) as xtp,
            tc.tile_pool(name="scales", bufs=3) as spool,
            tc.tile_pool(name="ypool", bufs=4) as ypool,
            tc.tile_pool(name="psum", bufs=8, space="PSUM") as psum,
        ):
            # ---- weight-block scales * 2 (undoes the host /2),
            # broadcast to all partitions: wsb[p, nb, kb] = 2*ws[nb, kb].
            ws_row = const.tile([1, NB * KB], f32)
            nc.sync.dma_start(
                ws_row[:], ws_d[:].rearrange("a b -> (a b)")[None, :]
            )
            wsb = const.tile([P, NB, KB], f32)
            nc.gpsimd.partition_broadcast(
                wsb[:].rearrange("p a b -> p (a b)"), ws_row[:]
            )

            # ---- weight prep: one contiguous DMA per k-slab (already
            # K-on-partitions), one dequant multiply to bf16. No PE work.
            # Dequants split DVE/GpSimd so neither engine's FIFO backlog
            # starves the first m-tiles' x-prep.
            wTs = [None] * KB

            # ---- x-prep for one m-tile: quantize+dequantize (two
            # k-halves), XBAR-transpose to K-on-partitions.
            def x_prep(mt):
                ms = slice(mt * P, (mt + 1) * P)
                xThalf = []
                for kh in range(2):
                    ks = slice(kh * KH * P, (kh + 1) * KH * P)
                    xrow = xpool.tile([P, KH, P], f32, name="xrow", tag="xrow")
                    nc.scalar.dma_start(
                        xrow[:],
                        x_d[ms, ks].rearrange("m (kb x) -> m kb x", x=P),
                    )
                    sc = spool.tile([P, 3, KH], f32, name="sc", tag="sc")
                    amax, rinv, s2 = sc[:, 0, :], sc[:, 1, :], sc[:, 2, :]
                    nc.vector.tensor_reduce(
                        amax, xrow[:], axis=mybir.AxisListType.X,
                        op=mybir.AluOpType.max, apply_absolute_value=True,
                    )
                    nc.vector.reciprocal(rinv, amax)
                    nc.vector.tensor_scalar_mul(rinv, rinv, float(FP8_SAFE))
                    nc.vector.tensor_scalar_mul(s2, amax, float(1.0 / FP8_SAFE))
                    xq = xpool.tile([P, KH, P], f8, name="xq", tag="xq")
                    nc.vector.tensor_tensor(
                        xq[:], xrow[:], rinv[:, :, None].to_broadcast((P, KH, P)),
                        mybir.AluOpType.mult,
                    )
                    xdq = xpool.tile([P, KH, P], bf16, name="xdq", tag="xdq")
                    nc.vector.tensor_tensor(
                        xdq[:], xq[:], s2[:, :, None].to_broadcast((P, KH, P)),
                        mybir.AluOpType.mult,
                    )
                    xT = xtp.tile([P, KH, P], bf16, name="xT", tag="xT")
                    nc.sync.dma_start_transpose(
                        xT[:], xdq[:].rearrange("p a b -> p (a b)")
                    )
                    xThalf.append(xT)
                return xThalf

            def drains(mt, pts):
                ms = slice(mt * P, (mt + 1) * P)
                for c in range(NCH):
                    yt = ypool.tile([P, CHW], bf16, name="yt", tag="yt")
                    nc.scalar.activation(
                        yt[:], pts[c][:], mybir.ActivationFunctionType.Copy
                    )
                    # y via SWDGE keeps HWDGE lanes clear for x loads +
                    # transposes.
                    nc.gpsimd.dma_start(y_d[ms, c * CHW:(c + 1) * CHW], yt[:])

            # ---- first m-tile's x-prep is emitted before the weight
            # loop so its loads aren't queued behind 32 w8 DMA triggers.
            xT_first = x_prep(0)
            for kb in range(KB):
                w8 = w8p.tile([P, NB, P], f8, name="w8", tag="w8")
                # sync queue: keeps the scalar engine's HWDGE ring free
                # for the per-m-tile x loads (32 queued triggers would
                # delay mt1+'s prep by ~20us).
                nc.sync.dma_start(
                    w8[:].rearrange("p a b -> p (a b)"),
                    wt8_d[kb * P:(kb + 1) * P, :],
                )
                wT = wtp.tile([P, NB, P], bf16, name="wT", tag=f"wT{kb}")
                on_gp = kb < 3 or (kb >= 9 and kb % 2 == 1)
                eng = nc.gpsimd if on_gp else nc.vector
                eng.tensor_tensor(
                    wT[:], w8[:],
                    wsb[:, :, kb, None].to_broadcast((P, NB, P)),
                    mybir.AluOpType.mult,
                )
                wTs[kb] = wT

            # ---- software-pipelined main loop: x-prep one m-tile ahead,
            # drains one m-tile behind (their matmul-completion waits are
            # then pre-satisfied and never block the scalar queue).
            xT_next = xT_first
            prev = None
            for mt in range(MT):
                xThalf = xT_next
                if mt + 1 < MT:
                    xT_next = x_prep(mt + 1)
                if prev is not None:
                    drains(*prev)
                pts = [
                    psum.tile([P, CHW], mybir.dt.float32, name=f"pt{c}", tag="pt")
                    for c in range(NCH)
                ]
                for kh in range(2):
                    for c in range(NCH):
                        for kb in range(KH):
                            wv = wTs[kh * KH + kb][:].rearrange("p a b -> p (a b)")
                            nc.tensor.matmul(
                                pts[c][:],
                                xThalf[kh][:, kb, :],
                                wv[:, c * CHW:(c + 1) * CHW],
                                start=(kh == 0 and kb == 0),
                                stop=(kh == 1 and kb == KH - 1),
                            )
                prev = (mt, pts)
            drains(*prev)

    nc.compile()
    return nc


def _core_inputs(x, weight, ws, c, nsh=NSH, nb=NB):
    """Shard + lay out inputs for core c. Layout/dtype transforms only:
    the fp8 cast of w/2 is exact (all values <= 224)."""
    import ml_dtypes

    wsl = weight[c * nsh:(c + 1) * nsh]
    wt8 = np.ascontiguousarray(
        (wsl.T * np.float32(0.5)).astype(ml_dtypes.float8_e4m3)
    )
    return {
        "x": x,
        "wt8": wt8,
        "ws": np.ascontiguousarray(ws[c * nb:(c + 1) * nb] * np.float32(2.0)),
    }


def kernel(x, weight, weight_scale_inv):
    from concourse.bass_utils import run_bass_kernel_spmd

    if "nc" not in _NC_CACHE:
        _NC_CACHE["nc"] = _build()
    nc = _NC_CACHE["nc"]

    x = np.ascontiguousarray(np.asarray(x, dtype=np.float32))
    weight = np.asarray(weight, dtype=np.float32)
    ws = np.asarray(weight_scale_inv, dtype=np.float32)

    in_maps = [_core_inputs(x, weight, ws, c) for c in range(NCORES)]
    res = run_bass_kernel_spmd(nc, in_maps, list(range(NCORES)))
    y = np.concatenate(
        [np.asarray(res.results[c]["y"]) for c in range(NCORES)], axis=1
    )
    return y.astype(np.float32, copy=False)
